# revision 11
# baseline (speedup 1.0000x reference)
"""Trainium2 Bass kernel for nn_Decouple (per-pixel dynamic 3x3 kernel with
dilation 2, then 3x3 conv + bias + LeakyReLU 0.2).

Sharding: pure data parallel over 8 cores; core c handles image n = c//2,
output rows [96*s, 96*s+96) with s = c%2. Inside each core the 96 rows are
split into two 48-row halves stacked on the 128 SBUF partitions
(partitions 0-63 = half A channels, 64-127 = half B channels).

All tensor data is bf16 (inputs quantized on host): halves HBM traffic,
doubles DVE throughput, and runs PE matmuls at 1 cycle/row.

Pipeline per tile (single y DMA per tile, [128, 9, rows, W]):
  DMA : 1 y-block load (gpsimd SWDGE), x rows JIT (scalar)
  DVE : 3 batched per-group products (overlapping-window x APs) +
        NLVL[t] batched tree pre-adds -> 9/6/3 tap streams
  PE  : identity matmuls accumulate the streams in PSUM (more streams on
        early DMA-bound tiles keep the PE p-state hot with useful work)
  ACT : PSUM -> SBUF out1 tile (+halo rows from prev tile)
  PE  : 3x3 conv = 9 block-diagonal matmuls into PSUM
  ACT : Prelu(conv + bias, alpha=0.2) -> SBUF bf16, DMA out (sync)
"""
import sys

if "/opt/trn_rl_repo" not in sys.path:
    sys.path.append("/opt/trn_rl_repo")

import json

import numpy as np
import ml_dtypes

import bass_rust
import concourse.bass as bass
import concourse.tile as tile
from concourse import mybir
from concourse.bass_utils import run_bass_kernel_spmd

F32 = mybir.dt.float32
BF16 = mybir.dt.bfloat16
NPBF = ml_dtypes.bfloat16

N, C, H, W = 4, 64, 192, 192
DIL = 2
N_CORES = 8
HS = H // 2          # rows per core (96)
HH = HS // 2         # rows per half (48)
OUT2 = [2, 4, 6, 8, 10, 10, 8]      # out rows per tile (sum = 48), ramped
T = len(OUT2)
A = [sum(OUT2[:i]) for i in range(T + 1)]  # tile start rows
NWARM = 22           # PE p-state warmup matmuls (ramp to 2.4GHz during y DMA)


def _tile_rows(t):
    # out1 rows computed in tile t
    return OUT2[0] + 2 if t == 0 else OUT2[t]


def _y_block_offsets():
    """(t, g) -> offset into the flat y-prep array (128-partition blocks)."""
    offs = {}
    off = 0
    for t in range(T):
        rows = _tile_rows(t)
        for g in range(3):
            offs[(t, g)] = off
            off += 2 * C * 3 * rows * W
    return offs, off


_Y_OFFS, _Y_TOTAL = _y_block_offsets()


def _legalize_waits(nc):
    """This container's walrus accepts at most ONE sync wait per instruction.
    Split any instruction with k>1 waits into k-1 single-wait NoOps inserted
    immediately before it on the same engine."""
    raw = json.loads(type(nc).to_json_bytes(nc))
    counter = [0]
    for func in raw.get("functions", []):
        for blk in func.get("blocks", []):
            new_insts = []
            for inst in blk.get("instructions", []):
                si = inst.get("sync_info")
                waits = (si or {}).get("on_wait") or []
                if len(waits) > 1:
                    for w in waits[:-1]:
                        counter[0] += 1
                        new_insts.append(
                            {
                                "engine": inst["engine"],
                                "ins": [],
                                "name": f"wsplit_{counter[0]}",
                                "opcode": "NoOp",
                                "outs": [],
                                "sync_info": {"on_update": [], "on_wait": [w]},
                            }
                        )
                    si["on_wait"] = [waits[-1]]
                new_insts.append(inst)
            blk["instructions"] = new_insts
    fixed = json.dumps(raw).encode()
    nc.to_json_bytes = lambda: fixed


def _xwin(xp, x0, rows):
    """Overlapping-window view of xp: [128, 3 (dj stride 2), rows, W]."""
    v = xp[:, x0 : x0 + rows, :].unsqueeze(1).broadcast_to(
        [128, 3, rows, W + 4]
    )
    ap = [list(p) for p in v.ap]
    ap[1] = [2, 3]          # dj: stride 2, 3 taps
    ap[3] = [1, W]          # crop cols to W
    v.ap = bass_rust.VecI64Pair(ap)
    return v


def build_nc():
    nc = bass.Bass()
    xin = nc.declare_dram_parameter("xin", [2 * C, HH + 6, W + 4], BF16, isOutput=False)
    yp = nc.declare_dram_parameter("yp", [_Y_TOTAL], BF16, isOutput=False)
    w9 = nc.declare_dram_parameter("w9", [9, 128, 128], BF16, isOutput=False)
    ident = nc.declare_dram_parameter("ident", [128, 128], BF16, isOutput=False)
    bias = nc.declare_dram_parameter("bias", [128, 1], F32, isOutput=False)
    out = nc.declare_dram_parameter("out", [2 * C, HH, W], BF16, isOutput=True)

    XROWS = HH + 6  # x rows per half (54)
    O1R = HH + 2    # out1 rows per half (50)

    with tile.TileContext(nc) as tc:
        with (
            tc.tile_pool(name="consts", bufs=1) as consts,
            tc.tile_pool(name="ypool", bufs=11) as ypool,
            tc.tile_pool(name="out2p", bufs=2) as out2p,
            tc.tile_pool(name="ps1", bufs=4, space="PSUM") as ps1,
            tc.tile_pool(name="ps2", bufs=3, space="PSUM") as ps2,
            tc.tile_pool(name="pswarm", bufs=1, space="PSUM") as pswarm,
        ):
            xp = consts.tile([128, XROWS, W + 4], BF16)

            # x rows are loaded just-in-time (sync-engine issue), one tile
            # ahead of use, so no big x transfer sits in front of y blocks
            xdone = [0]

            def load_x_for(t):
                if t < 0 or t >= T:
                    return
                if t == 0:
                    need = 4 + _tile_rows(0)
                else:
                    need = min(XROWS, A[t + 1] + 6)
                if need > xdone[0]:
                    nc.sync.dma_start(
                        xp[:, xdone[0] : need, :], xin[:, xdone[0] : need, :]
                    )
                    xdone[0] = need

            def load_y(t):
                rows = _tile_rows(t)
                ysl = []
                for g in range(3):
                    ys = ypool.tile(
                        [128, 3, rows, W], BF16, tag="y", name=f"ys_{t}_{g}"
                    )
                    off = _Y_OFFS[(t, g)]
                    blk = 2 * C * 3 * rows * W
                    src = yp[off : off + blk].rearrange("(c f) -> c f", c=2 * C)
                    dst = ys.rearrange("p a b c -> p (a b c)")
                    nc.gpsimd.dma_start(dst, src)
                    ysl.append(ys)
                return ysl

            # startup: warmsrc memset gates the PE warmup (no DMA dep);
            # ident + x + w9 on sync behind the first y blocks
            warmsrc = consts.tile([128, 512], BF16)
            nc.vector.memset(warmsrc[:], 0)

            id_sb = consts.tile([128, 128], BF16)
            nc.sync.dma_start(id_sb[:], ident[:])

            ybufs = [None] * T
            ybufs[0] = load_y(0)
            load_x_for(0)
            ybufs[1] = load_y(1)
            load_x_for(1)
            ybufs[2] = load_y(2)

            w_sb = consts.tile([128, 9, 128], BF16)
            nc.sync.dma_start(w_sb[:], w9.rearrange("t p m -> p t m"))
            b_sb = consts.tile([128, 1], F32)
            nc.sync.dma_start(b_sb[:], bias[:])

            # spin the PE while tile-0 y is in flight: the tensor engine
            # reaches full clock only after ~3us of continuous execution
            warm = pswarm.tile([128, 512], F32, tag="warm")
            for _ in range(NWARM):
                nc.tensor.matmul(
                    warm[:], warmsrc[:, 0:128], warmsrc[:], start=True, stop=True
                )

            # persistent out1 buffer: 50 rows x W; flat view for linear copies
            o1 = consts.tile([128, O1R, W], BF16)
            o1f = o1.rearrange("p r w -> p (r w)")

            for t in range(T):
                rows = _tile_rows(t)
                r2 = OUT2[t]
                it0 = 0 if t == 0 else A[t] + 2  # first o1 row this tile fills
                nfree = rows * W

                ysl = ybufs[t]
                if t + 3 < T:
                    ybufs[t + 3] = load_y(t + 3)
                load_x_for(t + 1)

                # ---- products: one batched op per tap group (in place),
                #      then per-group pair pre-add -> 2 streams per group ----
                for g in range(3):
                    x0 = (2 * g) if t == 0 else (A[t] + 2 * g + 2)
                    pv = ysl[g][:, :, :, :]
                    nc.vector.tensor_tensor(
                        pv, _xwin(xp, x0, rows), pv, op=mybir.AluOpType.mult
                    )
                    d = ysl[g][:, 0, :, :]
                    nc.vector.tensor_tensor(
                        d, d, ysl[g][:, 1, :, :], op=mybir.AluOpType.add
                    )

                # ---- tap-sum: 512-elem linear chunks -> PSUM -> o1 ----
                nstr = 6
                for c0 in range(0, nfree, 512):
                    cn = min(512, nfree - c0)
                    p1 = ps1.tile([128, 512], F32, tag="p1")
                    i = 0
                    for g in range(3):
                        for kk in (0, 2):
                            prf = ysl[g][:, kk, :, :].rearrange(
                                "p r w -> p (r w)"
                            )
                            nc.tensor.matmul(
                                p1[:, 0:cn],
                                id_sb[:],
                                prf[:, c0 : c0 + cn],
                                start=(i == 0),
                                stop=(i == nstr - 1),
                            )
                            i += 1
                    dst = o1f[:, it0 * W + c0 : it0 * W + c0 + cn]
                    nc.scalar.copy(dst, p1[:, 0:cn])

                # ---- conv for output row-chunks now fully available ----
                o2 = out2p.tile([128, r2, W], BF16, tag="out2")
                for j in range(A[t] // 2, (A[t] + r2) // 2):
                    p2 = ps2.tile([128, 2, W], F32, tag="p2")
                    for tp in (1, 0, 2, 3, 4, 5, 6, 7, 8):
                        ki, kj = divmod(tp, 3)
                        if kj == 0:
                            sc, dc, cw = 0, 1, W - 1
                        elif kj == 1:
                            sc, dc, cw = 0, 0, W
                        else:
                            sc, dc, cw = 1, 0, W - 1
                        nc.tensor.matmul(
                            p2[:, :, dc : dc + cw],
                            w_sb[:, tp, :],
                            o1[:, 2 * j + ki : 2 * j + ki + 2, sc : sc + cw],
                            start=(tp == 1),
                            stop=(tp == 8),
                        )
                    lr = 2 * j - A[t]
                    nc.scalar.activation(
                        o2[:, lr : lr + 2, :],
                        p2[:],
                        mybir.ActivationFunctionType.Prelu,
                        bias=b_sb[:, 0:1],
                        scale=1.0,
                        alpha=0.2,
                    )

                # ---- store: one 128-partition DMA per tile ----
                nc.sync.dma_start(out[:, A[t] : A[t] + r2, :], o2[:])
    _legalize_waits(nc)
    return nc


_NC_CACHE = None


def _get_nc():
    global _NC_CACHE
    if _NC_CACHE is None:
        _NC_CACHE = build_nc()
    return _NC_CACHE


def _prep_core_inputs(x, y, n, s):
    h0 = s * HS
    xpad = np.zeros((C, HS + 6, W + 4), dtype=np.float32)
    a, b = max(0, h0 - 3), min(H, h0 + HS + 3)
    xpad[:, a - (h0 - 3) : b - (h0 - 3), 2 : W + 2] = x[n][:, a:b, :]
    # stacked halves: [2, C, 54, W+4] -> [128, 54, W+4]
    XR = HH + 6
    xin = np.concatenate(
        [xpad[:, 0:XR, :], xpad[:, HH : HH + XR, :]], axis=0
    ).reshape(2 * C, XR, W + 4)

    # padded y rows [h0-1, h0+97), as [C, 9, 98, W]
    yin = np.zeros((C, 9, HS + 2, W), dtype=np.float32)
    a, b = max(0, h0 - 1), min(H, h0 + HS + 1)
    yin[:, :, a - (h0 - 1) : b - (h0 - 1), :] = y[n].reshape(C, 9, H, W)[
        :, :, a:b, :
    ]

    # per (tile, group): one contiguous [2C, 3, rows, W] block
    ypf = np.empty(_Y_TOTAL, dtype=np.float32)
    for t in range(T):
        rows = _tile_rows(t)
        r0 = 0 if t == 0 else A[t] + 2
        for g in range(3):
            off = _Y_OFFS[(t, g)]
            blk2 = 2 * C * 3 * rows * W
            arr = np.concatenate(
                [
                    yin[:, 3 * g : 3 * g + 3, r0 : r0 + rows, :],
                    yin[:, 3 * g : 3 * g + 3, r0 + HH : r0 + HH + rows, :],
                ],
                axis=0,
            )  # [2C, 3, rows, W]
            ypf[off : off + blk2] = arr.reshape(-1)
    return xin.astype(NPBF), ypf.astype(NPBF)


def _prep_weights(fuse_w, fuse_b):
    w9 = np.zeros((9, 128, 128), dtype=np.float32)
    for tp in range(9):
        ki, kj = divmod(tp, 3)
        wt = fuse_w[:, :, ki, kj].T  # [i, o]
        w9[tp, 0:64, 0:64] = wt
        w9[tp, 64:128, 64:128] = wt
    ident = np.eye(128, dtype=np.float32)
    bias = np.concatenate([fuse_b, fuse_b]).reshape(128, 1).astype(np.float32)
    return w9.astype(NPBF), ident.astype(NPBF), bias


def make_in_maps(x, y, fuse_w, fuse_b):
    w9, ident, bias = _prep_weights(fuse_w, fuse_b)
    in_maps = []
    for c in range(N_CORES):
        n, s = divmod(c, 2)
        xin, ypf = _prep_core_inputs(x, y, n, s)
        in_maps.append(
            {"xin": xin, "yp": ypf, "w9": w9, "ident": ident, "bias": bias}
        )
    return in_maps


def kernel(x, y, fuse_w, fuse_b):
    x = np.asarray(x, dtype=np.float32)
    y = np.asarray(y, dtype=np.float32)
    fuse_w = np.asarray(fuse_w, dtype=np.float32)
    fuse_b = np.asarray(fuse_b, dtype=np.float32)

    in_maps = make_in_maps(x, y, fuse_w, fuse_b)
    nc = _get_nc()
    res = run_bass_kernel_spmd(nc, in_maps, list(range(N_CORES)))

    full = np.empty((N, C, H, W), dtype=np.float32)
    for c in range(N_CORES):
        n, s = divmod(c, 2)
        o4 = np.asarray(res.results[c]["out"]).astype(np.float32)
        o4 = o4.reshape(2, C, HH, W)
        for half in range(2):
            r = s * HS + half * HH
            full[n, :, r : r + HH, :] = o4[half]
    return full


# revision 12
# speedup vs baseline: 1.2170x; 1.2170x over previous
"""Trainium2 Bass kernel for nn_Decouple (per-pixel dynamic 3x3 kernel with
dilation 2, then 3x3 conv + bias + LeakyReLU 0.2).

Sharding: pure data parallel over 8 cores; core c handles image n = c//2,
output rows [96*s, 96*s+96) with s = c%2. Inside each core the 96 rows are
split into two 48-row halves stacked on the 128 SBUF partitions
(partitions 0-63 = half A channels, 64-127 = half B channels).

All tensor data is bf16 (inputs quantized on host): halves HBM traffic,
doubles DVE throughput (2x_1p), and runs PE matmuls at 1 cycle/row
(vs ~2.4 for fp32 HIGH).

Pipeline per 12-row tile:
  DMA : 3 y-block loads (tap groups; 2 halves stacked on partitions)
  DVE : 9 per-tap products in place over y (bf16, 2x) + NPRE pair-adds
  PE  : (9-NPRE) identity matmuls accumulate taps in PSUM (bf16, K=128)
  ACT : PSUM -> padded SBUF out1 tile (+halo rows from prev tile)
  PE  : 3x3 conv = 9 block-diagonal matmuls into PSUM (bf16, M=128)
  ACT : Prelu(conv + bias, alpha=0.2) -> SBUF bf16, DMA out
"""
import sys

if "/opt/trn_rl_repo" not in sys.path:
    sys.path.append("/opt/trn_rl_repo")

import json

import numpy as np
import ml_dtypes

import concourse.bass as bass
import concourse.tile as tile
from concourse import mybir
from concourse.bass_utils import run_bass_kernel_spmd

F32 = mybir.dt.float32
BF16 = mybir.dt.bfloat16
NPBF = ml_dtypes.bfloat16

N, C, H, W = 4, 64, 192, 192
DIL = 2
N_CORES = 8
HS = H // 2          # rows per core (96)
HH = HS // 2         # rows per half (48)
OUT2 = [2, 12, 12, 12, 8, 2]     # out rows per tile (sum = 48)
T = len(OUT2)
A = [sum(OUT2[:i]) for i in range(T + 1)]  # tile start rows
RMAX = max(OUT2) + 2
NPRE = 3             # tap pairs pre-added on DVE (reduces PE tap streams)
NWARM = 22           # PE p-state warmup matmuls (ramp to 2.4GHz during y DMA)


def _tile_rows(t):
    # out1 rows computed in tile t
    return OUT2[0] + 2 if t == 0 else OUT2[t]


def _y_block_offsets():
    """(t, g) -> offset into the flat y-prep array (128-partition blocks)."""
    offs = {}
    off = 0
    for t in range(T):
        rows = _tile_rows(t)
        for g in range(3):
            offs[(t, g)] = off
            off += 2 * C * 3 * rows * W
    return offs, off


_Y_OFFS, _Y_TOTAL = _y_block_offsets()


def _legalize_waits(nc):
    """This container's walrus accepts at most ONE sync wait per instruction.
    Split any instruction with k>1 waits into k-1 single-wait NoOps inserted
    immediately before it on the same engine."""
    raw = json.loads(type(nc).to_json_bytes(nc))
    counter = [0]
    for func in raw.get("functions", []):
        for blk in func.get("blocks", []):
            new_insts = []
            for inst in blk.get("instructions", []):
                si = inst.get("sync_info")
                waits = (si or {}).get("on_wait") or []
                if len(waits) > 1:
                    for w in waits[:-1]:
                        counter[0] += 1
                        new_insts.append(
                            {
                                "engine": inst["engine"],
                                "ins": [],
                                "name": f"wsplit_{counter[0]}",
                                "opcode": "NoOp",
                                "outs": [],
                                "sync_info": {"on_update": [], "on_wait": [w]},
                            }
                        )
                    si["on_wait"] = [waits[-1]]
                new_insts.append(inst)
            blk["instructions"] = new_insts
    fixed = json.dumps(raw).encode()
    nc.to_json_bytes = lambda: fixed


# tap streams after NPRE pre-adds: pairs (kk0 += kk1) within groups 0..NPRE-1
def _streams():
    s = []
    for g in range(3):
        if g < NPRE:
            s += [(g, 0), (g, 2)]
        else:
            s += [(g, 0), (g, 1), (g, 2)]
    return s


_STREAMS = _streams()


def build_nc():
    nc = bass.Bass()
    xin = nc.declare_dram_parameter("xin", [2 * C, HH + 6, W + 4], BF16, isOutput=False)
    yp = nc.declare_dram_parameter("yp", [_Y_TOTAL], BF16, isOutput=False)
    w9 = nc.declare_dram_parameter("w9", [128, 9 * 128], BF16, isOutput=False)
    ident = nc.declare_dram_parameter("ident", [128, 128], BF16, isOutput=False)
    bias = nc.declare_dram_parameter("bias", [128, 1], F32, isOutput=False)
    out = nc.declare_dram_parameter("out", [2 * C, HH, W], BF16, isOutput=True)

    XROWS = HH + 6  # x rows per half (54)
    O1R = HH + 2    # out1 rows per half (50)

    with tile.TileContext(nc) as tc:
        with (
            tc.tile_pool(name="consts", bufs=1) as consts,
            tc.tile_pool(name="ypool", bufs=8) as ypool,
            tc.tile_pool(name="out2p", bufs=2) as out2p,
            tc.tile_pool(name="ps1", bufs=4, space="PSUM") as ps1,
            tc.tile_pool(name="ps2", bufs=3, space="PSUM") as ps2,
            tc.tile_pool(name="pswarm", bufs=1, space="PSUM") as pswarm,
        ):
            xp = consts.tile([128, XROWS, W + 4], BF16)

            # x rows are loaded just-in-time, interleaved between the y tile
            # loads, so no big x transfer sits in front of early y blocks
            xdone = [0]

            def load_x_for(t):
                if t == 0:
                    need = 4 + _tile_rows(0)
                else:
                    need = min(XROWS, A[t + 1] + 6)
                if need > xdone[0]:
                    nc.gpsimd.dma_start(
                        xp[:, xdone[0] : need, :], xin[:, xdone[0] : need, :]
                    )
                    xdone[0] = need

            def load_y(t):
                rows = _tile_rows(t)
                ysl = []
                for g in range(3):
                    ys = ypool.tile(
                        [128, 3, rows, W], BF16, tag="y", name=f"ys_{t}_{g}"
                    )
                    off = _Y_OFFS[(t, g)]
                    blk = 2 * C * 3 * rows * W
                    src = yp[off : off + blk].rearrange("(c f) -> c f", c=2 * C)
                    dst = ys.rearrange("p a b c -> p (a b c)")
                    nc.gpsimd.dma_start(dst, src)
                    ysl.append(ys)
                return ysl

            # PE warmup source: no DMA dependency, so the tensor engine can
            # start ramping to full clock as soon as the program starts
            warmsrc = consts.tile([128, 512], BF16)
            nc.vector.memset(warmsrc[:], 0)
            warm = pswarm.tile([128, 512], F32, tag="warm")
            for _ in range(NWARM):
                nc.tensor.matmul(
                    warm[:], warmsrc[:, 0:128], warmsrc[:], start=True, stop=True
                )

            load_x_for(0)
            y_t0 = load_y(0)

            id_sb = consts.tile([128, 128], BF16)
            nc.sync.dma_start(id_sb[:], ident[:])
            w_sb = consts.tile([128, 9, 128], BF16)
            nc.sync.dma_start(w_sb.rearrange("p t m -> p (t m)"), w9[:])
            b_sb = consts.tile([128, 1], F32)
            nc.sync.dma_start(b_sb[:], bias[:])

            # persistent out1 buffer: 50 rows x W, NO column padding (conv
            # uses column-cropped APs instead); flat view for linear copies
            o1 = consts.tile([128, O1R, W], BF16)
            o1f = o1.rearrange("p r w -> p (r w)")

            for t in range(T):
                rows = _tile_rows(t)
                r2 = OUT2[t]
                it0 = 0 if t == 0 else A[t] + 2  # first o1 row this tile fills
                nfree = rows * W

                if t == 0:
                    ysl = y_t0
                else:
                    load_x_for(t)
                    ysl = load_y(t)

                # ---- products (in place, bf16 2x) + NPRE pair pre-adds ----
                for k in range(9):
                    g, kk = divmod(k, 3)
                    di, dj = divmod(k, 3)
                    x0 = (2 * di) if t == 0 else (A[t] + 2 * di + 2)
                    pv = ysl[g][:, kk, :, :]
                    nc.vector.tensor_tensor(
                        pv,
                        xp[:, x0 : x0 + rows, 2 * dj : 2 * dj + W],
                        pv,
                        op=mybir.AluOpType.mult,
                    )
                    if g < NPRE and kk == 1:
                        p0 = ysl[g][:, 0, :, :]
                        nc.vector.tensor_tensor(
                            p0, p0, pv, op=mybir.AluOpType.add
                        )

                # ---- tap-sum: 512-elem linear chunks -> PSUM -> o1 ----
                nstr = len(_STREAMS)
                for c0 in range(0, nfree, 512):
                    cn = min(512, nfree - c0)
                    p1 = ps1.tile([128, 512], F32, tag="p1")
                    for i, (g, kk) in enumerate(_STREAMS):
                        prf = ysl[g][:, kk, :, :].rearrange("p r w -> p (r w)")
                        nc.tensor.matmul(
                            p1[:, 0:cn],
                            id_sb[:],
                            prf[:, c0 : c0 + cn],
                            start=(i == 0),
                            stop=(i == nstr - 1),
                        )
                    dst = o1f[:, it0 * W + c0 : it0 * W + c0 + cn]
                    nc.scalar.copy(dst, p1[:, 0:cn])

                # ---- conv for output row-chunks now fully available ----
                o2 = out2p.tile([128, r2, W], BF16, tag="out2")
                for j in range(A[t] // 2, (A[t] + r2) // 2):
                    p2 = ps2.tile([128, 2, W], F32, tag="p2")
                    for tp in (1, 0, 2, 3, 4, 5, 6, 7, 8):
                        ki, kj = divmod(tp, 3)
                        if kj == 0:
                            sc, dc, cw = 0, 1, W - 1
                        elif kj == 1:
                            sc, dc, cw = 0, 0, W
                        else:
                            sc, dc, cw = 1, 0, W - 1
                        nc.tensor.matmul(
                            p2[:, :, dc : dc + cw],
                            w_sb[:, tp, :],
                            o1[:, 2 * j + ki : 2 * j + ki + 2, sc : sc + cw],
                            start=(tp == 1),
                            stop=(tp == 8),
                        )
                    lr = 2 * j - A[t]
                    nc.scalar.activation(
                        o2[:, lr : lr + 2, :],
                        p2[:],
                        mybir.ActivationFunctionType.Prelu,
                        bias=b_sb[:, 0:1],
                        scale=1.0,
                        alpha=0.2,
                    )

                # ---- store: one 128-partition DMA per tile ----
                nc.sync.dma_start(out[:, A[t] : A[t] + r2, :], o2[:])
    _legalize_waits(nc)
    return nc


_NC_CACHE = None


def _get_nc():
    global _NC_CACHE
    if _NC_CACHE is None:
        _NC_CACHE = build_nc()
    return _NC_CACHE


def _prep_core_inputs(x, y, n, s):
    h0 = s * HS
    xpad = np.zeros((C, HS + 6, W + 4), dtype=np.float32)
    a, b = max(0, h0 - 3), min(H, h0 + HS + 3)
    xpad[:, a - (h0 - 3) : b - (h0 - 3), 2 : W + 2] = x[n][:, a:b, :]
    # stacked halves: [2, C, 54, W+4] -> [128, 54, W+4]
    XR = HH + 6
    xin = np.concatenate(
        [xpad[:, 0:XR, :], xpad[:, HH : HH + XR, :]], axis=0
    ).reshape(2 * C, XR, W + 4)

    # padded y rows [h0-1, h0+97), as [C, 9, 98, W]
    yin = np.zeros((C, 9, HS + 2, W), dtype=np.float32)
    a, b = max(0, h0 - 1), min(H, h0 + HS + 1)
    yin[:, :, a - (h0 - 1) : b - (h0 - 1), :] = y[n].reshape(C, 9, H, W)[
        :, :, a:b, :
    ]

    ypf = np.empty(_Y_TOTAL, dtype=np.float32)
    for t in range(T):
        rows = _tile_rows(t)
        r0 = 0 if t == 0 else A[t] + 2
        for g in range(3):
            off = _Y_OFFS[(t, g)]
            blk = C * 3 * rows * W
            for half in range(2):
                rr = r0 + HH * half
                ypf[off + half * blk : off + (half + 1) * blk] = yin[
                    :, 3 * g : 3 * g + 3, rr : rr + rows, :
                ].reshape(-1)
    return xin.astype(NPBF), ypf.astype(NPBF)


def _prep_weights(fuse_w, fuse_b):
    w9 = np.zeros((9, 128, 128), dtype=np.float32)
    for tp in range(9):
        ki, kj = divmod(tp, 3)
        wt = fuse_w[:, :, ki, kj].T  # [i, o]
        w9[tp, 0:64, 0:64] = wt
        w9[tp, 64:128, 64:128] = wt
    # pre-transpose to [p, t*m] so the device DMA is one contiguous
    # 2304B descriptor per partition (vs 1152 tiny 256B descriptors)
    w9 = np.ascontiguousarray(w9.transpose(1, 0, 2)).reshape(128, 9 * 128)
    ident = np.eye(128, dtype=np.float32)
    bias = np.concatenate([fuse_b, fuse_b]).reshape(128, 1).astype(np.float32)
    return w9.astype(NPBF), ident.astype(NPBF), bias


def make_in_maps(x, y, fuse_w, fuse_b):
    w9, ident, bias = _prep_weights(fuse_w, fuse_b)
    in_maps = []
    for c in range(N_CORES):
        n, s = divmod(c, 2)
        xin, ypf = _prep_core_inputs(x, y, n, s)
        in_maps.append(
            {"xin": xin, "yp": ypf, "w9": w9, "ident": ident, "bias": bias}
        )
    return in_maps


def kernel(x, y, fuse_w, fuse_b):
    x = np.asarray(x, dtype=np.float32)
    y = np.asarray(y, dtype=np.float32)
    fuse_w = np.asarray(fuse_w, dtype=np.float32)
    fuse_b = np.asarray(fuse_b, dtype=np.float32)

    in_maps = make_in_maps(x, y, fuse_w, fuse_b)
    nc = _get_nc()
    res = run_bass_kernel_spmd(nc, in_maps, list(range(N_CORES)))

    full = np.empty((N, C, H, W), dtype=np.float32)
    for c in range(N_CORES):
        n, s = divmod(c, 2)
        o4 = np.asarray(res.results[c]["out"]).astype(np.float32)
        o4 = o4.reshape(2, C, HH, W)
        for half in range(2):
            r = s * HS + half * HH
            full[n, :, r : r + HH, :] = o4[half]
    return full


# revision 16
# speedup vs baseline: 1.2192x; 1.0018x over previous
"""Trainium2 Bass kernel for nn_Decouple (per-pixel dynamic 3x3 kernel with
dilation 2, then 3x3 conv + bias + LeakyReLU 0.2).

Sharding: pure data parallel over 8 cores; core c handles image n = c//2,
output rows [96*s, 96*s+96) with s = c%2. Inside each core the 96 rows are
split into two 48-row halves stacked on the 128 SBUF partitions
(partitions 0-63 = half A channels, 64-127 = half B channels).

All tensor data is bf16 (inputs quantized on host): halves HBM traffic,
doubles DVE throughput (2x_1p), and runs PE matmuls at 1 cycle/row
(vs ~2.4 for fp32 HIGH).

Pipeline per 12-row tile:
  DMA : 3 y-block loads (tap groups; 2 halves stacked on partitions)
  DVE : 9 per-tap products in place over y (bf16, 2x) + NPRE pair-adds
  PE  : (9-NPRE) identity matmuls accumulate taps in PSUM (bf16, K=128)
  ACT : PSUM -> padded SBUF out1 tile (+halo rows from prev tile)
  PE  : 3x3 conv = 9 block-diagonal matmuls into PSUM (bf16, M=128)
  ACT : Prelu(conv + bias, alpha=0.2) -> SBUF bf16, DMA out
"""
import sys

if "/opt/trn_rl_repo" not in sys.path:
    sys.path.append("/opt/trn_rl_repo")

import json

import numpy as np
import ml_dtypes

import concourse.bass as bass
import concourse.tile as tile
from concourse import mybir
from concourse.bass_utils import run_bass_kernel_spmd

F32 = mybir.dt.float32
BF16 = mybir.dt.bfloat16
NPBF = ml_dtypes.bfloat16

N, C, H, W = 4, 64, 192, 192
DIL = 2
N_CORES = 8
HS = H // 2          # rows per core (96)
HH = HS // 2         # rows per half (48)
OUT2 = [2, 12, 12, 12, 8, 2]     # out rows per tile (sum = 48)
T = len(OUT2)
A = [sum(OUT2[:i]) for i in range(T + 1)]  # tile start rows
RMAX = max(OUT2) + 2
NPRE = 3             # tap pairs pre-added on DVE (reduces PE tap streams)


def _tile_rows(t):
    # out1 rows computed in tile t
    return OUT2[0] + 2 if t == 0 else OUT2[t]


def _y_block_offsets():
    """(t, g) -> offset into the flat y-prep array (128-partition blocks)."""
    offs = {}
    off = 0
    for t in range(T):
        rows = _tile_rows(t)
        for g in range(3):
            offs[(t, g)] = off
            off += 2 * C * 3 * rows * W
    return offs, off


_Y_OFFS, _Y_TOTAL = _y_block_offsets()


def _legalize_waits(nc):
    """This container's walrus accepts at most ONE sync wait per instruction.
    Split any instruction with k>1 waits into k-1 single-wait NoOps inserted
    immediately before it on the same engine."""
    raw = json.loads(type(nc).to_json_bytes(nc))
    counter = [0]
    for func in raw.get("functions", []):
        for blk in func.get("blocks", []):
            new_insts = []
            for inst in blk.get("instructions", []):
                si = inst.get("sync_info")
                waits = (si or {}).get("on_wait") or []
                if len(waits) > 1:
                    for w in waits[:-1]:
                        counter[0] += 1
                        new_insts.append(
                            {
                                "engine": inst["engine"],
                                "ins": [],
                                "name": f"wsplit_{counter[0]}",
                                "opcode": "NoOp",
                                "outs": [],
                                "sync_info": {"on_update": [], "on_wait": [w]},
                            }
                        )
                    si["on_wait"] = [waits[-1]]
                new_insts.append(inst)
            blk["instructions"] = new_insts
    fixed = json.dumps(raw).encode()
    nc.to_json_bytes = lambda: fixed


# tap streams after NPRE pre-adds: pairs (kk0 += kk1) within groups 0..NPRE-1
def _streams():
    s = []
    for g in range(3):
        if g < NPRE:
            s += [(g, 0), (g, 2)]
        else:
            s += [(g, 0), (g, 1), (g, 2)]
    return s


_STREAMS = _streams()


def build_nc():
    nc = bass.Bass()
    xin = nc.declare_dram_parameter("xin", [2 * C, HH + 6, W + 4], BF16, isOutput=False)
    yp = nc.declare_dram_parameter("yp", [_Y_TOTAL], BF16, isOutput=False)
    w9 = nc.declare_dram_parameter("w9", [128, 9 * 128], BF16, isOutput=False)
    ident = nc.declare_dram_parameter("ident", [128, 128], BF16, isOutput=False)
    bias = nc.declare_dram_parameter("bias", [128, 1], F32, isOutput=False)
    out = nc.declare_dram_parameter("out", [2 * C, HH, W], BF16, isOutput=True)

    XROWS = HH + 6  # x rows per half (54)
    O1R = HH + 2    # out1 rows per half (50)

    with tile.TileContext(nc) as tc:
        with (
            tc.tile_pool(name="consts", bufs=1) as consts,
            tc.tile_pool(name="ypool", bufs=10) as ypool,
            tc.tile_pool(name="out2p", bufs=2) as out2p,
            tc.tile_pool(name="ps1", bufs=4, space="PSUM") as ps1,
            tc.tile_pool(name="ps2", bufs=4, space="PSUM") as ps2,
        ):
            xp = consts.tile([128, XROWS, W + 4], BF16)

            # x rows are loaded just-in-time, interleaved between the y tile
            # loads, so no big x transfer sits in front of early y blocks
            xdone = [0]

            def load_x_for(t):
                if t == 0:
                    need = 4 + _tile_rows(0)
                else:
                    need = min(XROWS, A[t + 1] + 6)
                if need > xdone[0]:
                    nc.gpsimd.dma_start(
                        xp[:, xdone[0] : need, :], xin[:, xdone[0] : need, :]
                    )
                    xdone[0] = need

            def load_y(t):
                rows = _tile_rows(t)
                ysl = []
                for g in range(3):
                    ys = ypool.tile(
                        [128, 3, rows, W], BF16, tag="y", name=f"ys_{t}_{g}"
                    )
                    off = _Y_OFFS[(t, g)]
                    blk = 2 * C * 3 * rows * W
                    src = yp[off : off + blk].rearrange("(c f) -> c f", c=2 * C)
                    dst = ys.rearrange("p a b c -> p (a b c)")
                    nc.gpsimd.dma_start(dst, src)
                    ysl.append(ys)
                return ysl

            load_x_for(0)
            y_t0 = load_y(0)

            id_sb = consts.tile([128, 128], BF16)
            nc.sync.dma_start(id_sb[:], ident[:])
            w_sb = consts.tile([128, 9, 128], BF16)
            nc.sync.dma_start(w_sb.rearrange("p t m -> p (t m)"), w9[:])
            b_sb = consts.tile([128, 1], F32)
            nc.sync.dma_start(b_sb[:], bias[:])

            # persistent out1 buffer: 50 rows x W, NO column padding (conv
            # uses column-cropped APs instead); flat view for linear copies
            o1 = consts.tile([128, O1R, W], BF16)
            o1f = o1.rearrange("p r w -> p (r w)")

            def emit_tapsum_chunk(ysl, it0, c0, cn):
                nstr = len(_STREAMS)
                p1 = ps1.tile([128, 512], F32, tag="p1")
                for i, (g, kk) in enumerate(_STREAMS):
                    prf = ysl[g][:, kk, :, :].rearrange("p r w -> p (r w)")
                    nc.tensor.matmul(
                        p1[:, 0:cn],
                        id_sb[:],
                        prf[:, c0 : c0 + cn],
                        start=(i == 0),
                        stop=(i == nstr - 1),
                    )
                dst = o1f[:, it0 * W + c0 : it0 * W + c0 + cn]
                nc.scalar.copy(dst, p1[:, 0:cn])

            def emit_conv_j(o2, tm1, j):
                p2 = ps2.tile([128, 2, W], F32, tag="p2")
                for tp in (1, 0, 2, 3, 4, 5, 6, 7, 8):
                    ki, kj = divmod(tp, 3)
                    if kj == 0:
                        sc, dc, cw = 0, 1, W - 1
                    elif kj == 1:
                        sc, dc, cw = 0, 0, W
                    else:
                        sc, dc, cw = 1, 0, W - 1
                    nc.tensor.matmul(
                        p2[:, :, dc : dc + cw],
                        w_sb[:, tp, :],
                        o1[:, 2 * j + ki : 2 * j + ki + 2, sc : sc + cw],
                        start=(tp == 1),
                        stop=(tp == 8),
                    )
                lr = 2 * j - A[tm1]
                nc.scalar.activation(
                    o2[:, lr : lr + 2, :],
                    p2[:],
                    mybir.ActivationFunctionType.Prelu,
                    bias=b_sb[:, 0:1],
                    scale=1.0,
                    alpha=0.2,
                )

            # software pipeline at one-tile offset: PE interleaves the
            # tap-sum chunks of tile t with the conv row-pairs of tile t-1,
            # so the conv never waits on this tile's PSUM->o1 copies
            for t in range(T + 1):
                chunks = []
                if t < T:
                    rows = _tile_rows(t)
                    it0 = 0 if t == 0 else A[t] + 2
                    nfree = rows * W

                    if t == 0:
                        ysl = y_t0
                    else:
                        load_x_for(t)
                        ysl = load_y(t)

                    # -- products (in place) + NPRE pair pre-adds (DVE) --
                    for k in range(9):
                        g, kk = divmod(k, 3)
                        di, dj = divmod(k, 3)
                        x0 = (2 * di) if t == 0 else (A[t] + 2 * di + 2)
                        pv = ysl[g][:, kk, :, :]
                        nc.vector.tensor_tensor(
                            pv,
                            xp[:, x0 : x0 + rows, 2 * dj : 2 * dj + W],
                            pv,
                            op=mybir.AluOpType.mult,
                        )
                        if g < NPRE and kk == 1:
                            p0 = ysl[g][:, 0, :, :]
                            nc.vector.tensor_tensor(
                                p0, p0, pv, op=mybir.AluOpType.add
                            )
                    chunks = [
                        (c0, min(512, nfree - c0))
                        for c0 in range(0, nfree, 512)
                    ]

                js = []
                o2 = None
                if t >= 1:
                    r2p = OUT2[t - 1]
                    o2 = out2p.tile([128, r2p, W], BF16, tag="out2")
                    js = list(range(A[t - 1] // 2, (A[t - 1] + r2p) // 2))

                ci, ji = 0, 0
                nc_, nj = len(chunks), len(js)
                while ci < nc_ or ji < nj:
                    if ji < nj and (ci >= nc_ or ji * nc_ <= ci * nj):
                        emit_conv_j(o2, t - 1, js[ji])
                        ji += 1
                    else:
                        c0, cn = chunks[ci]
                        emit_tapsum_chunk(ysl, it0, c0, cn)
                        ci += 1

                if t >= 1:
                    # ---- store: one 128-partition DMA per tile ----
                    nc.sync.dma_start(
                        out[:, A[t - 1] : A[t - 1] + OUT2[t - 1], :], o2[:]
                    )
    _legalize_waits(nc)
    return nc


_NC_CACHE = None


def _get_nc():
    global _NC_CACHE
    if _NC_CACHE is None:
        _NC_CACHE = build_nc()
    return _NC_CACHE


def _prep_core_inputs(x, y, n, s):
    h0 = s * HS
    xpad = np.zeros((C, HS + 6, W + 4), dtype=np.float32)
    a, b = max(0, h0 - 3), min(H, h0 + HS + 3)
    xpad[:, a - (h0 - 3) : b - (h0 - 3), 2 : W + 2] = x[n][:, a:b, :]
    # stacked halves: [2, C, 54, W+4] -> [128, 54, W+4]
    XR = HH + 6
    xin = np.concatenate(
        [xpad[:, 0:XR, :], xpad[:, HH : HH + XR, :]], axis=0
    ).reshape(2 * C, XR, W + 4)

    # padded y rows [h0-1, h0+97), as [C, 9, 98, W]
    yin = np.zeros((C, 9, HS + 2, W), dtype=np.float32)
    a, b = max(0, h0 - 1), min(H, h0 + HS + 1)
    yin[:, :, a - (h0 - 1) : b - (h0 - 1), :] = y[n].reshape(C, 9, H, W)[
        :, :, a:b, :
    ]

    ypf = np.empty(_Y_TOTAL, dtype=np.float32)
    for t in range(T):
        rows = _tile_rows(t)
        r0 = 0 if t == 0 else A[t] + 2
        for g in range(3):
            off = _Y_OFFS[(t, g)]
            blk = C * 3 * rows * W
            for half in range(2):
                rr = r0 + HH * half
                ypf[off + half * blk : off + (half + 1) * blk] = yin[
                    :, 3 * g : 3 * g + 3, rr : rr + rows, :
                ].reshape(-1)
    return xin.astype(NPBF), ypf.astype(NPBF)


def _prep_weights(fuse_w, fuse_b):
    w9 = np.zeros((9, 128, 128), dtype=np.float32)
    for tp in range(9):
        ki, kj = divmod(tp, 3)
        wt = fuse_w[:, :, ki, kj].T  # [i, o]
        w9[tp, 0:64, 0:64] = wt
        w9[tp, 64:128, 64:128] = wt
    # pre-transpose to [p, t*m] so the device DMA is one contiguous
    # 2304B descriptor per partition (vs 1152 tiny 256B descriptors)
    w9 = np.ascontiguousarray(w9.transpose(1, 0, 2)).reshape(128, 9 * 128)
    ident = np.eye(128, dtype=np.float32)
    bias = np.concatenate([fuse_b, fuse_b]).reshape(128, 1).astype(np.float32)
    return w9.astype(NPBF), ident.astype(NPBF), bias


def make_in_maps(x, y, fuse_w, fuse_b):
    w9, ident, bias = _prep_weights(fuse_w, fuse_b)
    in_maps = []
    for c in range(N_CORES):
        n, s = divmod(c, 2)
        xin, ypf = _prep_core_inputs(x, y, n, s)
        in_maps.append(
            {"xin": xin, "yp": ypf, "w9": w9, "ident": ident, "bias": bias}
        )
    return in_maps


def kernel(x, y, fuse_w, fuse_b):
    x = np.asarray(x, dtype=np.float32)
    y = np.asarray(y, dtype=np.float32)
    fuse_w = np.asarray(fuse_w, dtype=np.float32)
    fuse_b = np.asarray(fuse_b, dtype=np.float32)

    in_maps = make_in_maps(x, y, fuse_w, fuse_b)
    nc = _get_nc()
    res = run_bass_kernel_spmd(nc, in_maps, list(range(N_CORES)))

    full = np.empty((N, C, H, W), dtype=np.float32)
    for c in range(N_CORES):
        n, s = divmod(c, 2)
        o4 = np.asarray(res.results[c]["out"]).astype(np.float32)
        o4 = o4.reshape(2, C, HH, W)
        for half in range(2):
            r = s * HS + half * HH
            full[n, :, r : r + HH, :] = o4[half]
    return full
